# revision 27
# baseline (speedup 1.0000x reference)
"""Causal self-attention (B=4, T=2048, D=1024, 16 heads) on 8 TRN2 NeuronCores.

Sharding: tensor-parallel over heads — each core owns 2 heads (a 128-dim slice
of the QKV projections, column-parallel) and the matching 128 rows of W_O
(row-parallel). Each core computes a full-shape partial output; the host sums
the 8 partials.

v2: all SBUF operands and DRAM I/O in bf16 (halves DMA + DVE traffic, enables
FWL weight loads; PSUM accumulation stays fp32), causal mask applied by
preloading -100 into the diagonal PSUM blocks with a [128,128] matmul
(removes the exp->DVE-mask->PV cross-engine dependency), V' ones-columns
written once into two persistent ping-pong buffers, O-proj PSUM evacuation
split between VectorE and ScalarE.

Per-core dataflow (all matmuls bf16 in / fp32 PSUM accumulate):
  x.T [1024, 8192] (host-transposed bf16, streamed in 512-token chunks)
  Q.T/K.T = W.T-slice @ x.T          -> [128, 2048] per batch (d-major)
  V       = x-chunk.T @ W_V.T-slice  -> [tok, 128] tiles into persistent V'
            buffers [128 keys, 2*(64+1)] (ones column yields softmax sums for
            free in the PV matmul)
  S.T     = K-block @ Q.T-chunk      -> [128 keys, <=512 queries] per block,
            two heads row-packed in the 128-wide PE array (K=64 each);
            diagonal blocks start from a -100 upper-triangular preload matmul
  P.T     = exp(S.T / 8)  (ScalarE; no max-subtraction: scores ~ N(0,1))
  out.T   = V'.T @ P.T accumulated over key blocks -> [65, 512] PSUM
            (row 64 = softmax sums)
  normalize: recip(sums) -> partition-broadcast -> multiply
  out_partial.T = W_O-slice.T-chunk @ attnout    -> [1024, 8192] bf16 out

Projection matmuls for batch b+1 are interleaved between attention positions
of batch b (the attention inner loop is ACT-bound; dense interleaved PE work
keeps the tensor engine HAM-warm at 2.4 GHz).
"""
import os
import numpy as np
import ml_dtypes
import concourse.bacc as bacc
import concourse.mybir as mybir
import concourse.tile as tile
from concourse import bass_utils

B, T, D = 4, 2048, 1024
NH, DH = 16, 64
NC = 8
HPC = NH // NC        # 2 heads per core
CS = HPC * DH         # 128 projection dims per core
TOK = B * T           # 8192 tokens
QC = 512              # query-chunk width
NCH = T // QC         # 4 chunks per batch
KT = D // 128         # 8 contraction tiles
NKB = T // 128        # 16 key blocks per batch
f32 = mybir.dt.float32
bf16 = mybir.dt.bfloat16
AFT = mybir.ActivationFunctionType
SCALE = float(1.0 / np.sqrt(DH))
NV = DH + 1           # 65: V head columns + ones column
NPBF = ml_dtypes.bfloat16

_cache = {}


def _build():
    if "nc" in _cache:
        return _cache["nc"]
    nc = bacc.Bacc("TRN2", target_bir_lowering=False, debug=False)

    xT_d = nc.dram_tensor("xT", [D, TOK], bf16, kind="ExternalInput").ap()
    WQT_d = nc.dram_tensor("WQT", [D, CS], bf16, kind="ExternalInput").ap()
    WKT_d = nc.dram_tensor("WKT", [D, CS], bf16, kind="ExternalInput").ap()
    WVT_d = nc.dram_tensor("WVT", [D, CS], bf16, kind="ExternalInput").ap()
    WOT_d = nc.dram_tensor("WOT", [CS, D], bf16, kind="ExternalInput").ap()
    mprel_d = nc.dram_tensor("mprel", [128, 128], bf16, kind="ExternalInput").ap()
    ident_d = nc.dram_tensor("ident", [128, 128], bf16, kind="ExternalInput").ap()
    ones_d = nc.dram_tensor("onesc", [128, 2], bf16, kind="ExternalInput").ap()
    out_d = nc.dram_tensor("outT", [D, TOK], bf16, kind="ExternalOutput").ap()

    with tile.TileContext(nc) as tc:
      with nc.allow_low_precision(reason="bf16 attention"):
        with tc.tile_pool(name="sb", bufs=1) as sb, \
             tc.tile_pool(name="sp", bufs=2) as sp, \
             tc.tile_pool(name="ps", bufs=1, space="PSUM") as ps:
            # ---- constants / weights (persistent)
            WQT_t = sb.tile([128, KT * CS], bf16, tag="wqt")
            WKT_t = sb.tile([128, KT * CS], bf16, tag="wkt")
            WVT_t = sb.tile([128, KT * CS], bf16, tag="wvt")
            WOT_t = sb.tile([128, D], bf16, tag="wot")
            mprel_t = sb.tile([128, 128], bf16, tag="mprel")
            ident_t = sb.tile([128, 128], bf16, tag="ident")
            ones_t = sb.tile([128, 2], bf16, tag="ones")
            vpbuf = [sb.tile([128, NKB * 2 * NV], bf16, tag=f"vp{p}", name=f"vpbuf{p}") for p in range(2)]

            def load_weights():
                # emitted AFTER the first x-load DMAs so x0 heads its queue
                for k in range(KT):
                    nc.scalar.dma_start(out=WQT_t[:, k * CS:(k + 1) * CS], in_=WQT_d[k * 128:(k + 1) * 128, :])
                    nc.scalar.dma_start(out=WKT_t[:, k * CS:(k + 1) * CS], in_=WKT_d[k * 128:(k + 1) * 128, :])
                    nc.gpsimd.dma_start(out=WVT_t[:, k * CS:(k + 1) * CS], in_=WVT_d[k * 128:(k + 1) * 128, :])
                nc.gpsimd.dma_start(out=WOT_t[:], in_=WOT_d[:, :])
                nc.scalar.dma_start(out=mprel_t[:], in_=mprel_d[:, :])
                nc.scalar.dma_start(out=ident_t[:], in_=ident_d[:, :])
                nc.scalar.dma_start(out=ones_t[:], in_=ones_d[:, :])
                # persistent V' ping-pong buffers (by batch parity); ones
                # columns written once, V-projection only writes [0:DH] parts
                for p in range(2):
                    dst = vpbuf[p][:].rearrange("p (kb h x) -> p kb h x", kb=NKB, h=2)[:, :, :, DH:NV]
                    src = ones_t[:].rearrange("p (o h x) -> p o h x", o=1, h=2).broadcast_to([128, NKB, 2, 1])
                    nc.vector.tensor_copy(dst, src)

            qt = {}   # per-batch Q.T [128, T] bf16
            kt = {}   # per-batch K.T [128, T]

            def vp_w(b, kb):
                """V' write view for (batch, key block): [128, 2, DH]."""
                base = kb * 2 * NV
                return vpbuf[b % 2][:, base:base + 2 * NV].rearrange("p (h x) -> p h x", h=2)[:, :, 0:DH]

            def vp_r(b, kb, h):
                """V' read view for PV matmul: [128, NV]."""
                base = kb * 2 * NV + h * NV
                return vpbuf[b % 2][:, base:base + NV]

            def proj_steps(b, ch):
                """QKV projection for chunk ch of batch b as emit-closures, so the
                matmuls can be interleaved between attention positions."""
                g = NCH * b + ch
                if ch == 0:
                    qt[b] = sp.tile([128, T], bf16, tag="qt", name=f"qt{b}", bufs=2)
                    kt[b] = sp.tile([128, T], bf16, tag="kt", name=f"kt{b}", bufs=2)
                xts = {}

                def load_x():
                    xtile = sp.tile([128, KT * QC], bf16, tag="xt", name=f"xt_{g}", bufs=4)
                    src = xT_d[:, g * QC:(g + 1) * QC].rearrange("(kb p) q -> p kb q", p=128)
                    dst = xtile[:].rearrange("p (kb q) -> p kb q", kb=KT)
                    nc.sync.dma_start(out=dst, in_=src)
                    xts[0] = xtile
                steps = [load_x]

                def xs(k, lo=0, hi=QC):
                    return xts[0][:, k * QC + lo:k * QC + hi]

                for wt, dst, nm in ((WQT_t, qt[b], "q"), (WKT_t, kt[b], "k")):
                    pp = ps.tile([128, QC], f32, tag="mm", name=f"pp{nm}{g}", bufs=2)
                    for k0 in range(0, KT, 2):
                        def fqk(wt=wt, dst=dst, k0=k0, pp=pp, ch=ch):
                            for k in (k0, k0 + 1):
                                nc.tensor.matmul(pp[:], wt[:, k * CS:(k + 1) * CS], xs(k),
                                                 start=(k == 0), stop=(k == KT - 1))
                            if k0 + 2 == KT:
                                nc.vector.tensor_copy(dst[:, ch * QC:(ch + 1) * QC], pp[:])
                        steps.append(fqk)

                for tt in range(QC // 128):
                    kb = ch * (QC // 128) + tt
                    vpp = ps.tile([128, CS], f32, tag="mm", name=f"vpp{g}_{tt}", bufs=2)
                    for k0 in range(0, KT, 4):
                        def fv(tt=tt, k0=k0, kb=kb, vpp=vpp):
                            for k in range(k0, k0 + 4):
                                nc.tensor.matmul(vpp[:], xs(k, tt * 128, (tt + 1) * 128),
                                                 WVT_t[:, k * CS:(k + 1) * CS],
                                                 start=(k == 0), stop=(k == KT - 1))
                            if k0 + 4 == KT:
                                src2 = vpp[:].rearrange("p (h x) -> p h x", h=2)
                                nc.vector.tensor_copy(vp_w(b, kb), src2)
                        steps.append(fv)
                return steps

            pending = []  # queued proj closures, interleaved into attention

            def pull(n):
                for _ in range(min(n, len(pending))):
                    pending.pop(0)()

            def oproj_steps(g, ao):
                """O-projection for chunk g as filler steps (2 output tiles each).
                PSUM evacuation alternates VectorE / ScalarE; one batched
                3D-strided DMA stores the whole chunk."""
                steps = []
                ott = {}
                for mt in range(8):
                    def fo(mt=mt):
                        if mt == 0:
                            ott[0] = sp.tile([128, 8 * QC], bf16, tag="ot", name=f"ot{g}", bufs=2)
                        op = ps.tile([128, QC], f32, tag="mm", name=f"op{g}_{mt}", bufs=2)
                        nc.tensor.matmul(op[:], WOT_t[:, mt * 128:(mt + 1) * 128], ao[:],
                                         start=True, stop=True)
                        ot = ott[0][:, mt * QC:(mt + 1) * QC]
                        nc.vector.tensor_copy(ot, op[:])
                        if mt == 7:
                            dst = out_d[:, g * QC:(g + 1) * QC].rearrange("(mt p) q -> p mt q", p=128)
                            src = ott[0][:].rearrange("p (mt q) -> p mt q", mt=8)
                            # gpsimd queue: keeps the store's copy-wait off the
                            # sync queue so x-load DMAs are never blocked; all
                            # copies on vector so the wait is short
                            nc.gpsimd.dma_start(out=dst, in_=src)
                    steps.append(fo)
                return steps

            def attn_chunk(b, ch, oproj_prev):
                """Attention + normalize for query chunk ch of batch b.
                Two-stage software pipeline: scores/exp for kb+1 are issued before
                the PV matmuls of kb, so the PV weight-loads never wait on exp.
                oproj_prev = (g, ao) of the previous chunk, interleaved here."""
                g = NCH * b + ch
                pvs = [ps.tile([65, QC], f32, tag=f"pv{h}", name=f"pv{h}_{g}", bufs=1) for h in range(HPC)]
                nkb = 4 * ch + 4

                def scores(kb):
                    off = max(0, 128 * kb - QC * ch)
                    diag = 128 * kb >= QC * ch
                    sc = ps.tile([128, 2 * QC], f32, tag="sc", name=f"sc{g}_{kb}", bufs=2)
                    pt = sp.tile([128, 2 * QC], bf16, tag="pt", name=f"pt{g}_{kb}", bufs=4)
                    for h in range(HPC):
                        hb = h * QC
                        if diag:  # preload -100 above the diagonal, scores accumulate on top
                            nc.tensor.matmul(sc[:, hb + off:hb + off + 128],
                                             mprel_t[:], ident_t[:],
                                             start=True, stop=False)
                        nc.tensor.matmul(sc[:, hb + off:hb + QC],
                                         kt[b][h * DH:(h + 1) * DH, kb * 128:(kb + 1) * 128],
                                         qt[b][h * DH:(h + 1) * DH, ch * QC + off:(ch + 1) * QC],
                                         start=not diag, stop=True)
                    if off == 0:
                        nc.scalar.activation(pt[:], sc[:], AFT.Exp, scale=SCALE)
                    else:
                        sc3 = sc[:].rearrange("p (h x) -> p h x", h=2)[:, :, off:QC]
                        pt3e = pt[:].rearrange("p (h x) -> p h x", h=2)[:, :, off:QC]
                        nc.scalar.activation(pt3e, sc3, AFT.Exp, scale=SCALE)
                    return pt, off

                def pv_mm(kb, pt, off):
                    for h in range(HPC):
                        hb = h * QC
                        nc.tensor.matmul(pvs[h][:, off:QC],
                                         vp_r(b, kb, h),
                                         pt[:, hb + off:hb + QC],
                                         start=(kb == 0), stop=(kb == nkb - 1))

                if oproj_prev is not None:
                    # not at the front: an O-proj matmul pulled before the
                    # producing normalize-mul finishes would head-of-line
                    # block the PE queue; in the last batch park it at the
                    # back so the filler-starved tail chunks stay dense
                    ins = len(pending) if b == B - 1 else min(6, len(pending))
                    pending[ins:ins] = oproj_steps(*oproj_prev)
                q0 = scores(0)
                pull(3)
                q1 = scores(1)
                pull(3)
                for kb in range(2, nkb):
                    cur = scores(kb)
                    pull(2)
                    pv_mm(kb - 2, *q0)
                    q0, q1 = q1, cur
                pull(1)
                pv_mm(nkb - 2, *q0)
                pull(1)
                pv_mm(nkb - 1, *q1)
                # normalize -> attnout [128, 512] bf16; recips first so the
                # gpsimd broadcasts + pv-releasing multiplies start earliest
                ao = sp.tile([128, QC], bf16, tag="ao", name=f"ao{g}", bufs=4)
                rs = []
                for h in range(HPC):
                    s_h = sp.tile([1, QC], f32, tag="sh", name=f"sh{g}_{h}", bufs=3)
                    r_h = sp.tile([1, QC], f32, tag="rh", name=f"rh{g}_{h}", bufs=3)
                    nc.vector.tensor_copy(s_h[0:1, :], pvs[h][64:65, :])
                    nc.vector.reciprocal_approx_fast(out=r_h[0:1, :], in_=s_h[0:1, :])
                    rs.append(r_h)
                for h in range(HPC):
                    bc = sp.tile([DH, QC], f32, tag="bc", name=f"bc{g}_{h}", bufs=3)
                    nc.gpsimd.partition_broadcast(bc[:], rs[h][0:1, :])
                    nc.vector.tensor_mul(ao[h * DH:(h + 1) * DH, :], pvs[h][0:DH, :], bc[:])
                return (g, ao)

            # emission: uniform software pipeline — attention chunk i runs with
            # the projection matmuls of a later chunk (and an x-load DMA two
            # chunks out) interleaved as PE filler, so every stretch of the
            # timeline including the last batch has dense tensor-engine work.
            # The last batch's attention runs (3,1),(3,2),(3,3),(3,0) so the
            # filler-starved tail ends on the smallest (least exp-bound) chunk.
            seq_proj = [(b, ch) for b in range(B) for ch in range(NCH)]
            seq_attn = list(seq_proj)
            seq_attn[4 * (B - 1):] = [(B - 1, c) for c in (1, 2, 3, 0)]
            packs = {}

            def ensure_pack(p):
                if 0 <= p < len(seq_proj) and p not in packs:
                    st = proj_steps(*seq_proj[p])
                    packs[p] = (st[0], st[1:])
            for p in range(3):
                ensure_pack(p)
            packs[0][0]()   # x-load chunk 0 — heads the sync DMA queue
            packs[1][0]()   # prefetch x chunk 1
            load_weights()
            for s in packs[0][1]:
                s()         # proj chunk 0 upfront
            pending.append(packs[2][0])
            pending.extend(packs[1][1])
            oprev = None
            for i, (b, ch) in enumerate(seq_attn):
                ensure_pack(i + 3)
                if i + 3 in packs:
                    pending.append(packs[i + 3][0])
                if i + 2 in packs:
                    pending.extend(packs[i + 2][1])
                oprev = attn_chunk(b, ch, oprev)
                pull(len(pending))  # all deps of the next chunk emitted
            for s in oproj_steps(*oprev):
                s()

    nc.compile()
    _cache["nc"] = nc
    return nc


def kernel(x, W_Q, W_K, W_V, W_O):
    nc = _build()
    xT = np.ascontiguousarray(
        np.asarray(x, dtype=np.float32).reshape(TOK, D).T).astype(NPBF)
    mprel = (-100.0 * np.triu(np.ones((128, 128), dtype=np.float32), 1)).astype(NPBF)
    ident = np.eye(128, dtype=np.float32).astype(NPBF)
    onesc = np.ones((128, 2), dtype=np.float32).astype(NPBF)
    in_maps = []
    for c in range(NC):
        cs = slice(c * CS, (c + 1) * CS)
        in_maps.append({
            "xT": xT,
            "WQT": np.ascontiguousarray(np.asarray(W_Q, dtype=np.float32)[cs].T).astype(NPBF),
            "WKT": np.ascontiguousarray(np.asarray(W_K, dtype=np.float32)[cs].T).astype(NPBF),
            "WVT": np.ascontiguousarray(np.asarray(W_V, dtype=np.float32)[cs].T).astype(NPBF),
            "WOT": np.ascontiguousarray(np.asarray(W_O, dtype=np.float32)[:, cs].T).astype(NPBF),
            "mprel": mprel, "ident": ident, "onesc": onesc,
        })
    trace = bool(os.environ.get("KERNEL_TRACE"))
    res = bass_utils.run_bass_kernel_spmd(nc, in_maps, list(range(NC)), trace=trace)
    kernel.last_result = res
    out = np.zeros((D, TOK), dtype=np.float64)
    for c in range(NC):
        out += res.results[c]["outT"].astype(np.float64)
    return np.ascontiguousarray(out.T.reshape(B, T, D)).astype(np.float32)


# revision 31
# speedup vs baseline: 1.0282x; 1.0282x over previous
"""Causal self-attention (B=4, T=2048, D=1024, 16 heads) on 8 TRN2 NeuronCores.

Sharding: tensor-parallel over heads — each core owns 2 heads (a 128-dim slice
of the QKV projections, column-parallel) and the matching 128 rows of W_O
(row-parallel). Each core computes a full-shape partial output; the host sums
the 8 partials.

v2: all SBUF operands and DRAM I/O in bf16 (halves DMA + DVE traffic, enables
FWL weight loads; PSUM accumulation stays fp32), causal mask applied by
preloading -100 into the diagonal PSUM blocks with a [128,128] matmul
(removes the exp->DVE-mask->PV cross-engine dependency), V' ones-columns
written once into two persistent ping-pong buffers, O-proj PSUM evacuation
split between VectorE and ScalarE.

Per-core dataflow (all matmuls bf16 in / fp32 PSUM accumulate):
  x.T [1024, 8192] (host-transposed bf16, streamed in 512-token chunks)
  Q.T/K.T = W.T-slice @ x.T          -> [128, 2048] per batch (d-major)
  V       = x-chunk.T @ W_V.T-slice  -> [tok, 128] tiles into persistent V'
            buffers [128 keys, 2*(64+1)] (ones column yields softmax sums for
            free in the PV matmul)
  S.T     = K-block @ Q.T-chunk      -> [128 keys, <=512 queries] per block,
            two heads row-packed in the 128-wide PE array (K=64 each);
            diagonal blocks start from a -100 upper-triangular preload matmul
  P.T     = exp(S.T / 8)  (ScalarE; no max-subtraction: scores ~ N(0,1))
  out.T   = V'.T @ P.T accumulated over key blocks -> [65, 512] PSUM
            (row 64 = softmax sums)
  normalize: recip(sums) -> partition-broadcast -> multiply
  out_partial.T = W_O-slice.T-chunk @ attnout    -> [1024, 8192] bf16 out

Projection matmuls for batch b+1 are interleaved between attention positions
of batch b (the attention inner loop is ACT-bound; dense interleaved PE work
keeps the tensor engine HAM-warm at 2.4 GHz).
"""
import os
import numpy as np
import ml_dtypes
import concourse.bacc as bacc
import concourse.mybir as mybir
import concourse.tile as tile
from concourse import bass_utils

B, T, D = 4, 2048, 1024
NH, DH = 16, 64
NC = 8
HPC = NH // NC        # 2 heads per core
CS = HPC * DH         # 128 projection dims per core
TOK = B * T           # 8192 tokens
QC = 512              # query-chunk width
NCH = T // QC         # 4 chunks per batch
KT = D // 128         # 8 contraction tiles
NKB = T // 128        # 16 key blocks per batch
f32 = mybir.dt.float32
bf16 = mybir.dt.bfloat16
AFT = mybir.ActivationFunctionType
SCALE = float(1.0 / np.sqrt(DH))
NV = DH + 1           # 65: V head columns + ones column
NPBF = ml_dtypes.bfloat16

_cache = {}


def _build():
    if "nc" in _cache:
        return _cache["nc"]
    nc = bacc.Bacc("TRN2", target_bir_lowering=False, debug=False)

    xT_d = nc.dram_tensor("xT", [D, TOK], bf16, kind="ExternalInput").ap()
    WQT_d = nc.dram_tensor("WQT", [D, CS], bf16, kind="ExternalInput").ap()
    WKT_d = nc.dram_tensor("WKT", [D, CS], bf16, kind="ExternalInput").ap()
    WVT_d = nc.dram_tensor("WVT", [D, CS], bf16, kind="ExternalInput").ap()
    WOT_d = nc.dram_tensor("WOT", [CS, D], bf16, kind="ExternalInput").ap()
    mprel_d = nc.dram_tensor("mprel", [128, 128], bf16, kind="ExternalInput").ap()
    ident_d = nc.dram_tensor("ident", [128, 128], bf16, kind="ExternalInput").ap()
    ones_d = nc.dram_tensor("onesc", [128, 2], bf16, kind="ExternalInput").ap()
    out_d = nc.dram_tensor("outT", [D, TOK], bf16, kind="ExternalOutput").ap()

    with tile.TileContext(nc) as tc:
      with nc.allow_low_precision(reason="bf16 attention"):
        with tc.tile_pool(name="sb", bufs=1) as sb, \
             tc.tile_pool(name="sp", bufs=2) as sp, \
             tc.tile_pool(name="ps", bufs=1, space="PSUM") as ps:
            # ---- constants / weights (persistent)
            WQT_t = sb.tile([128, KT * CS], bf16, tag="wqt")
            WKT_t = sb.tile([128, KT * CS], bf16, tag="wkt")
            WVT_t = sb.tile([128, KT * CS], bf16, tag="wvt")
            for k in range(KT):
                nc.scalar.dma_start(out=WQT_t[:, k * CS:(k + 1) * CS], in_=WQT_d[k * 128:(k + 1) * 128, :])
                nc.scalar.dma_start(out=WKT_t[:, k * CS:(k + 1) * CS], in_=WKT_d[k * 128:(k + 1) * 128, :])
                nc.gpsimd.dma_start(out=WVT_t[:, k * CS:(k + 1) * CS], in_=WVT_d[k * 128:(k + 1) * 128, :])
            WOT_t = sb.tile([128, D], bf16, tag="wot")
            nc.gpsimd.dma_start(out=WOT_t[:], in_=WOT_d[:, :])
            mprel_t = sb.tile([128, 128], bf16, tag="mprel")
            nc.scalar.dma_start(out=mprel_t[:], in_=mprel_d[:, :])
            ident_t = sb.tile([128, 128], bf16, tag="ident")
            nc.scalar.dma_start(out=ident_t[:], in_=ident_d[:, :])
            ones_t = sb.tile([128, 2], bf16, tag="ones")
            nc.scalar.dma_start(out=ones_t[:], in_=ones_d[:, :])

            # persistent V' ping-pong buffers (by batch parity); ones columns
            # written once here, V-projection only ever writes the [0:DH] parts
            vpbuf = [sb.tile([128, NKB * 2 * NV], bf16, tag=f"vp{p}", name=f"vpbuf{p}") for p in range(2)]
            for p in range(2):
                dst = vpbuf[p][:].rearrange("p (kb h x) -> p kb h x", kb=NKB, h=2)[:, :, :, DH:NV]
                src = ones_t[:].rearrange("p (o h x) -> p o h x", o=1, h=2).broadcast_to([128, NKB, 2, 1])
                nc.vector.tensor_copy(dst, src)

            qt = {}   # per-batch Q.T [128, T] bf16
            kt = {}   # per-batch K.T [128, T]

            def vp_w(b, kb):
                """V' write view for (batch, key block): [128, 2, DH]."""
                base = kb * 2 * NV
                return vpbuf[b % 2][:, base:base + 2 * NV].rearrange("p (h x) -> p h x", h=2)[:, :, 0:DH]

            def vp_r(b, kb, h):
                """V' read view for PV matmul: [128, NV]."""
                base = kb * 2 * NV + h * NV
                return vpbuf[b % 2][:, base:base + NV]

            def proj_steps(b, ch):
                """QKV projection for chunk ch of batch b as emit-closures, so the
                matmuls can be interleaved between attention positions."""
                g = NCH * b + ch
                if ch == 0:
                    qt[b] = sp.tile([128, T], bf16, tag="qt", name=f"qt{b}", bufs=2)
                    kt[b] = sp.tile([128, T], bf16, tag="kt", name=f"kt{b}", bufs=2)
                xts = {}

                def load_x():
                    xtile = sp.tile([128, KT * QC], bf16, tag="xt", name=f"xt_{g}", bufs=3)
                    src = xT_d[:, g * QC:(g + 1) * QC].rearrange("(kb p) q -> p kb q", p=128)
                    dst = xtile[:].rearrange("p (kb q) -> p kb q", kb=KT)
                    nc.sync.dma_start(out=dst, in_=src)
                    xts[0] = xtile
                steps = [load_x]

                def xs(k, lo=0, hi=QC):
                    return xts[0][:, k * QC + lo:k * QC + hi]

                for wt, dst, nm in ((WQT_t, qt[b], "q"), (WKT_t, kt[b], "k")):
                    pp = ps.tile([128, QC], f32, tag="mm", name=f"pp{nm}{g}", bufs=2)
                    for k0 in range(0, KT, 2):
                        def fqk(wt=wt, dst=dst, k0=k0, pp=pp, ch=ch):
                            for k in (k0, k0 + 1):
                                nc.tensor.matmul(pp[:], wt[:, k * CS:(k + 1) * CS], xs(k),
                                                 start=(k == 0), stop=(k == KT - 1))
                            if k0 + 2 == KT:
                                nc.vector.tensor_copy(dst[:, ch * QC:(ch + 1) * QC], pp[:])
                        steps.append(fqk)

                for tt in range(QC // 128):
                    kb = ch * (QC // 128) + tt
                    vpp = ps.tile([128, CS], f32, tag="mm", name=f"vpp{g}_{tt}", bufs=2)
                    for k0 in range(0, KT, 4):
                        def fv(tt=tt, k0=k0, kb=kb, vpp=vpp):
                            for k in range(k0, k0 + 4):
                                nc.tensor.matmul(vpp[:], xs(k, tt * 128, (tt + 1) * 128),
                                                 WVT_t[:, k * CS:(k + 1) * CS],
                                                 start=(k == 0), stop=(k == KT - 1))
                            if k0 + 4 == KT:
                                src2 = vpp[:].rearrange("p (h x) -> p h x", h=2)
                                nc.vector.tensor_copy(vp_w(b, kb), src2)
                        steps.append(fv)
                return steps

            pending = []  # queued proj closures, interleaved into attention

            def pull(n):
                for _ in range(min(n, len(pending))):
                    pending.pop(0)()

            def oproj_steps(g, ao):
                """O-projection for chunk g as filler steps (2 output tiles each).
                PSUM evacuation alternates VectorE / ScalarE; one batched
                3D-strided DMA stores the whole chunk."""
                steps = []
                ott = {}
                for mt in range(8):
                    def fo(mt=mt):
                        if mt == 0:
                            ott[0] = sp.tile([128, 8 * QC], bf16, tag="ot", name=f"ot{g}", bufs=2)
                        op = ps.tile([128, QC], f32, tag="mm", name=f"op{g}_{mt}", bufs=2)
                        nc.tensor.matmul(op[:], WOT_t[:, mt * 128:(mt + 1) * 128], ao[:],
                                         start=True, stop=True)
                        ot = ott[0][:, mt * QC:(mt + 1) * QC]
                        if mt % 4 == 3:
                            nc.scalar.copy(out=ot, in_=op[:])
                        else:
                            nc.vector.tensor_copy(ot, op[:])
                        if mt == 7:
                            dst = out_d[:, g * QC:(g + 1) * QC].rearrange("(mt p) q -> p mt q", p=128)
                            src = ott[0][:].rearrange("p (mt q) -> p mt q", mt=8)
                            nc.gpsimd.dma_start(out=dst, in_=src)
                    steps.append(fo)
                return steps

            def attn_chunk(b, ch, oproj_prev):
                """Attention + normalize for query chunk ch of batch b.
                Two-stage software pipeline: scores/exp for kb+1 are issued before
                the PV matmuls of kb, so the PV weight-loads never wait on exp.
                oproj_prev = (g, ao) of the previous chunk, interleaved here."""
                g = NCH * b + ch
                pvs = [ps.tile([65, QC], f32, tag=f"pv{h}", name=f"pv{h}_{g}", bufs=1) for h in range(HPC)]
                nkb = 4 * ch + 4

                def scores(kb):
                    off = max(0, 128 * kb - QC * ch)
                    diag = 128 * kb >= QC * ch
                    sc = ps.tile([128, 2 * QC], f32, tag="sc", name=f"sc{g}_{kb}", bufs=2)
                    pt = sp.tile([128, 2 * QC], bf16, tag="pt", name=f"pt{g}_{kb}", bufs=4)
                    for h in range(HPC):
                        hb = h * QC
                        if diag:  # preload -100 above the diagonal, scores accumulate on top
                            nc.tensor.matmul(sc[:, hb + off:hb + off + 128],
                                             mprel_t[:], ident_t[:],
                                             start=True, stop=False)
                        nc.tensor.matmul(sc[:, hb + off:hb + QC],
                                         kt[b][h * DH:(h + 1) * DH, kb * 128:(kb + 1) * 128],
                                         qt[b][h * DH:(h + 1) * DH, ch * QC + off:(ch + 1) * QC],
                                         start=not diag, stop=True)
                    if off == 0:
                        nc.scalar.activation(pt[:], sc[:], AFT.Exp, scale=SCALE)
                    else:
                        sc3 = sc[:].rearrange("p (h x) -> p h x", h=2)[:, :, off:QC]
                        pt3e = pt[:].rearrange("p (h x) -> p h x", h=2)[:, :, off:QC]
                        nc.scalar.activation(pt3e, sc3, AFT.Exp, scale=SCALE)
                    return pt, off

                def pv_mm(kb, pt, off):
                    for h in range(HPC):
                        hb = h * QC
                        nc.tensor.matmul(pvs[h][:, off:QC],
                                         vp_r(b, kb, h),
                                         pt[:, hb + off:hb + QC],
                                         start=(kb == 0), stop=(kb == nkb - 1))

                if oproj_prev is not None:
                    # not at the front: an O-proj matmul pulled before the
                    # producing normalize-mul finishes would head-of-line
                    # block the PE queue; in the last batch park it at the
                    # back so the filler-starved tail chunks stay dense
                    ins = len(pending) if b == B - 1 else min(6, len(pending))
                    pending[ins:ins] = oproj_steps(*oproj_prev)
                q0 = scores(0)
                pull(3)
                q1 = scores(1)
                pull(3)
                for kb in range(2, nkb):
                    cur = scores(kb)
                    pull(2)
                    pv_mm(kb - 2, *q0)
                    q0, q1 = q1, cur
                pull(1)
                pv_mm(nkb - 2, *q0)
                pull(1)
                pv_mm(nkb - 1, *q1)
                # normalize -> attnout [128, 512] bf16; recips first so the
                # gpsimd broadcasts + pv-releasing multiplies start earliest
                ao = sp.tile([128, QC], bf16, tag="ao", name=f"ao{g}", bufs=4)
                rs = []
                for h in range(HPC):
                    s_h = sp.tile([1, QC], f32, tag="sh", name=f"sh{g}_{h}", bufs=3)
                    r_h = sp.tile([1, QC], f32, tag="rh", name=f"rh{g}_{h}", bufs=3)
                    nc.vector.tensor_copy(s_h[0:1, :], pvs[h][64:65, :])
                    nc.vector.reciprocal_approx_fast(out=r_h[0:1, :], in_=s_h[0:1, :])
                    rs.append(r_h)
                for h in range(HPC):
                    bc = sp.tile([DH, QC], f32, tag="bc", name=f"bc{g}_{h}", bufs=3)
                    nc.gpsimd.partition_broadcast(bc[:], rs[h][0:1, :])
                    nc.vector.tensor_mul(ao[h * DH:(h + 1) * DH, :], pvs[h][0:DH, :], bc[:])
                return (g, ao)

            # emission: uniform software pipeline — attention chunk i runs with
            # the projection matmuls of chunk i+1 (and the x-load DMA of chunk
            # i+2) interleaved as PE filler, so every stretch of the timeline
            # including the last batch has dense tensor-engine work
            seq = [(b, ch) for b in range(B) for ch in range(NCH)]
            packs = {}

            def ensure_pack(i):
                if 0 <= i < len(seq) and i not in packs:
                    st = proj_steps(*seq[i])
                    packs[i] = (st[0], st[1:])
            ensure_pack(0)
            ensure_pack(1)
            packs[0][0]()   # x-load chunk 0
            packs[1][0]()   # prefetch x chunk 1
            for s in packs[0][1]:
                s()         # proj chunk 0 upfront
            oprev = None
            for idx, (b, ch) in enumerate(seq):
                ensure_pack(idx + 2)
                if idx + 2 in packs:
                    pending.append(packs[idx + 2][0])
                if idx + 1 in packs:
                    pending.extend(packs[idx + 1][1])
                oprev = attn_chunk(b, ch, oprev)
                pull(len(pending))  # all deps of the next chunk emitted
            for s in oproj_steps(*oprev):
                s()

    nc.compile()
    _cache["nc"] = nc
    return nc


def kernel(x, W_Q, W_K, W_V, W_O):
    nc = _build()
    xT = np.ascontiguousarray(
        np.asarray(x, dtype=np.float32).reshape(TOK, D).T).astype(NPBF)
    mprel = (-100.0 * np.triu(np.ones((128, 128), dtype=np.float32), 1)).astype(NPBF)
    ident = np.eye(128, dtype=np.float32).astype(NPBF)
    onesc = np.ones((128, 2), dtype=np.float32).astype(NPBF)
    in_maps = []
    for c in range(NC):
        cs = slice(c * CS, (c + 1) * CS)
        in_maps.append({
            "xT": xT,
            "WQT": np.ascontiguousarray(np.asarray(W_Q, dtype=np.float32)[cs].T).astype(NPBF),
            "WKT": np.ascontiguousarray(np.asarray(W_K, dtype=np.float32)[cs].T).astype(NPBF),
            "WVT": np.ascontiguousarray(np.asarray(W_V, dtype=np.float32)[cs].T).astype(NPBF),
            "WOT": np.ascontiguousarray(np.asarray(W_O, dtype=np.float32)[:, cs].T).astype(NPBF),
            "mprel": mprel, "ident": ident, "onesc": onesc,
        })
    trace = bool(os.environ.get("KERNEL_TRACE"))
    res = bass_utils.run_bass_kernel_spmd(nc, in_maps, list(range(NC)), trace=trace)
    kernel.last_result = res
    out = np.zeros((D, TOK), dtype=np.float64)
    for c in range(NC):
        out += res.results[c]["outT"].astype(np.float64)
    return np.ascontiguousarray(out.T.reshape(B, T, D)).astype(np.float32)
